# revision 5
# baseline (speedup 1.0000x reference)
"""Trainium2 Bass kernel for nn_CombinedLoss (surface loss + Tversky loss).

The reference's 4D EDT over (C,D,H,W) collapses analytically (a zero
channel-neighbor always exists at distance 1), giving exactly

  dist_maps[:, 1] == (argmax_c probs != 1)

so the loss reduces to elementwise work + global reductions:

  surface = mean(p1 * ind),  ind = 0.5*(1 + sign(max(p0,p2) - p1))
            (ties from quantization count 1/2 -- matches the unbiased
             comparison of the full-precision values to ~4e-5 rel err)
  tversky = 1 - (tp + 1) / (0.5*(sum(p)+sum(t)) + 1),  tp = sum(p*t)

All inputs ship as fp8-e4m3 (exact for one-hot t; 3.6e-05 total rel err
host-validated for p), halving HBM wire bytes vs bf16. Work per core:

  * DMA: 9 transfers spread over 3 queues (SP-HWDGE, ACT-HWDGE, Pool-SWDGE),
    ~530KB/queue, since each queue streams ~135 GB/s.
  * DVE: per chunk m = max(p0,p2); sub = m - p1 (bf16). Same-engine RAW
    hazards need explicit sem chains (DVE pipelines without interlocks).
  * ACT: d = Sign(sub) -> fp8 {-1,0,+1}; Sign(0)=0 handles ties.
  * PE (diagonal trick, stationary = [127 p-cols | ones], moving = [t|ones]):
      psa0 += p_tile(c=0,2)^T @ [t|1]:  diag=tp02, col128=p-col-sums,
                                        row127=t-col-sums (st)
      psa1 += p_tile(c=1)^T @ [t|1]:    diag=tp1, col128=sum(p1), row127=st1
      psc  += p_tile(c=1)^T @ d_tile:   diag=p1*d
    The sacrificed col-127 ("lost") products are patched by 4 tiny DVE
    STT-accumulate ops over host-staged aux = [p_lost | t_lost].
  * Out: DVE copies psa0|psa1|psc into one [128,390] f32 row-block, single
    DMA out, no completion wait (the Block-exit drain covers it). Host does
    the final ~50K-element reduce and the 8-core scalar all-reduce.
"""

import numpy as np
import ml_dtypes

import concourse.bass as bass
import concourse.mybir as mybir
from concourse.bass_utils import run_bass_kernel_spmd

N_CORES = 8
B, C, D, H, W = 2, 3, 64, 128, 128
N_VOX = B * D * H * W            # 2_097_152
VOX_PER_CORE = N_VOX // N_CORES  # 262_144
P = 128
NCH = 4                          # chunks per core
CW = VOX_PER_CORE // (P * NCH)   # 512 cols per chunk per channel
TPC = CW // P                    # 4 tiles per chunk per channel
NT = NCH * TPC                   # 16 tiles per channel total
PW = C * TPC * P                 # 1536 p cols per chunk ([127p|1] tiles)
TW = C * TPC * (P + 1)           # 1548 t cols per chunk ([128t|1] tiles)
NLOST = C * NT                   # 48 lost cols
FINW = 129 + 129 + P + 4         # 390 packed output cols
N_WARM = 10

F8 = ml_dtypes.float8_e4m3fn

_CACHE = {}

# DVE chunk processing order == p-chunk arrival order (see queue plan below)
DVE_ORDER = [0, 2, 1, 3]
# PE psa chunk order == t-chunk arrival order
PSA_ORDER = [0, 1, 2, 3]


def _build_module():
    from contextlib import ExitStack

    Alu = mybir.AluOpType
    Act = mybir.ActivationFunctionType
    f32 = mybir.dt.float32
    bf16 = mybir.dt.bfloat16
    f8 = mybir.dt.float8e4

    nc = bass.Bass()
    p_in = nc.dram_tensor("p", [NCH, P, PW], f8, kind="ExternalInput")
    t_in = nc.dram_tensor("t", [NCH, P, TW], f8, kind="ExternalInput")
    aux_in = nc.dram_tensor("aux", [P, 2 * NLOST], f8, kind="ExternalInput")
    fin_out = nc.dram_tensor("fin", [P, FINW], f32, kind="ExternalOutput")

    with (
        ExitStack() as ctx,
        nc.sbuf_tensor([P, NCH * PW], f8) as p_sb,
        nc.sbuf_tensor([P, NCH * TW], f8) as t_sb,
        nc.sbuf_tensor([P, 2 * NLOST], f8) as aux_sb,
        nc.sbuf_tensor([P, NCH * CW], f8) as m_sb,
        nc.sbuf_tensor([P, NCH * CW], bf16) as sub_sb,
        nc.sbuf_tensor([P, NCH * CW], f8) as d_sb,
        nc.sbuf_tensor([P, NT], f8) as ml_sb,
        nc.sbuf_tensor([P, NT], bf16) as subl_sb,
        nc.sbuf_tensor([P, NT], f8) as dl_sb,
        nc.sbuf_tensor([P, NLOST], bf16) as junk_sb,
        nc.sbuf_tensor([P, P], f8) as warm_sb,
        nc.sbuf_tensor([P, 1], f32) as dum_sb,
        nc.sbuf_tensor([P, FINW], f32) as fin_sb,
        nc.psum_tensor([P, P + 1], f32) as psa0,
        nc.psum_tensor([P, P + 1], f32) as psa1,
        nc.psum_tensor([P, P], f32) as psc,
        nc.psum_tensor([P, P], f32) as psw,
        nc.Block(no_gpsimd_drain=True) as block,
    ):
        g_sem = ctx.enter_context(nc.semaphore("g_sem"))
        aux_sem = ctx.enter_context(nc.semaphore("aux_sem"))
        v_sem = ctx.enter_context(nc.semaphore("v_sem"))
        alr_sem = ctx.enter_context(nc.semaphore("alr_sem"))
        ar_sem = ctx.enter_context(nc.semaphore("ar_sem"))
        ad_sem = ctx.enter_context(nc.semaphore("ad_sem"))
        pe_sem = ctx.enter_context(nc.semaphore("pe_sem"))
        c_sem = ctx.enter_context(nc.semaphore("c_sem"))
        o_sem = ctx.enter_context(nc.semaphore("o_sem"))
        p_sems = [ctx.enter_context(nc.semaphore(f"p_sem{i}")) for i in range(NCH)]
        t_sems = [ctx.enter_context(nc.semaphore(f"t_sem{i}")) for i in range(NCH)]

        def pblk(ch, c):  # [128, 512] channel block of a p chunk
            off = ch * PW + c * CW
            return p_sb[:, off : off + CW]

        def ptile(ch, c, i):  # [128, 128] stationary tile ([127p|1])
            off = ch * PW + c * CW + i * P
            return p_sb[:, off : off + P]

        def tblk(ch, c, i):  # [128, 129] moving tile ([128t|1])
            off = ch * TW + c * TPC * (P + 1) + i * (P + 1)
            return t_sb[:, off : off + P + 1]

        def dtile(ch, i):  # [128, 128] moving d tile
            off = ch * CW + i * P
            return d_sb[:, off : off + P]

        p_lost = aux_sb[:, 0:NLOST]
        t_lost = aux_sb[:, NLOST : 2 * NLOST]

        # queue plan: sync p0,p1,t2 | scalar p2,p3,t3 | pool aux,t0,t1
        @block.sync
        def _(sync):
            sync.dma_start(p_sb[:, 0:PW], p_in[0]).then_inc(p_sems[0], 16)
            sync.dma_start(p_sb[:, PW : 2 * PW], p_in[1]).then_inc(p_sems[1], 16)
            sync.dma_start(
                t_sb[:, 2 * TW : 3 * TW], t_in[2]
            ).then_inc(t_sems[2], 16)
            sync.wait_ge(c_sem, 1)
            sync.dma_start(fin_out[:], fin_sb[:]).then_inc(o_sem, 16)

        @block.gpsimd
        def _(gpsimd):
            gpsimd.memset(warm_sb[:], 0.0).then_inc(g_sem, 1)
            gpsimd.dma_start(aux_sb[:], aux_in[:]).then_inc(aux_sem, 16)
            gpsimd.dma_start(t_sb[:, 0:TW], t_in[0]).then_inc(t_sems[0], 16)
            gpsimd.dma_start(t_sb[:, TW : 2 * TW], t_in[1]).then_inc(t_sems[1], 16)

        @block.vector
        def _(vector):
            nv = 0  # v_sem value after each op completes

            def inc(op):
                nonlocal nv
                nv += 1
                return op.then_inc(v_sem, 1)

            vector.wait_ge(aux_sem, 16)
            # lost-col patches: tp, sp, sum(p1) partials
            inc(vector.scalar_tensor_tensor(
                junk_sb[:], p_lost, 0.0, t_lost, Alu.bypass, Alu.mult,
                accum_out=fin_sb[:, 386:387],
            ))
            inc(vector.scalar_tensor_tensor(
                junk_sb[:], p_lost, 0.0, p_lost, Alu.bypass, Alu.max,
                accum_out=fin_sb[:, 387:388],
            ))
            inc(vector.scalar_tensor_tensor(
                junk_sb[:, 0:NT], p_lost[:, NT : 2 * NT], 0.0,
                p_lost[:, NT : 2 * NT], Alu.bypass, Alu.max,
                accum_out=fin_sb[:, 388:389],
            ))
            # lost-col indicator path
            inc(vector.tensor_tensor(
                ml_sb[:], p_lost[:, 0:NT], p_lost[:, 2 * NT : 3 * NT], Alu.max
            ))
            vector.wait_ge(v_sem, nv)
            inc(vector.tensor_tensor(
                subl_sb[:], ml_sb[:], p_lost[:, NT : 2 * NT], Alu.subtract
            ))
            vector.wait_ge(v_sem, nv)
            vector.sem_inc(alr_sem, 1)
            # main chunks, in p-arrival order
            for k, ch in enumerate(DVE_ORDER):
                vector.wait_ge(p_sems[ch], 16)
                o = ch * CW
                inc(vector.tensor_tensor(
                    m_sb[:, o : o + CW], pblk(ch, 0), pblk(ch, 2), Alu.max
                ))
                vector.wait_ge(v_sem, nv)
                inc(vector.tensor_tensor(
                    sub_sb[:, o : o + CW], m_sb[:, o : o + CW], pblk(ch, 1),
                    Alu.subtract,
                ))
                vector.wait_ge(v_sem, nv)
                vector.sem_inc(ar_sem, 1)
            # lost-col s1 patch (needs all signs incl sign_l)
            vector.wait_ge(ad_sem, NCH + 1)
            inc(vector.scalar_tensor_tensor(
                junk_sb[:, 0:NT], p_lost[:, NT : 2 * NT], 0.0, dl_sb[:],
                Alu.bypass, Alu.mult, accum_out=fin_sb[:, 389:390],
            ))
            # pack psums
            vector.wait_ge(pe_sem, 1)
            inc(vector.tensor_copy(fin_sb[:, 0:129], psa0[:]))
            inc(vector.tensor_copy(fin_sb[:, 129:258], psa1[:]))
            vector.wait_ge(pe_sem, 2)
            vector.wait_ge(v_sem, nv)
            vector.tensor_copy(fin_sb[:, 258:386], psc[:]).then_inc(c_sem, 1)

        @block.scalar
        def _(scalar):
            scalar.dma_start(
                p_sb[:, 2 * PW : 3 * PW], p_in[2]
            ).then_inc(p_sems[2], 16)
            scalar.dma_start(
                p_sb[:, 3 * PW : 4 * PW], p_in[3]
            ).then_inc(p_sems[3], 16)
            scalar.dma_start(
                t_sb[:, 3 * TW : 4 * TW], t_in[3]
            ).then_inc(t_sems[3], 16)
            # dummy Sign pulls the ACT table load forward, off the critical path
            scalar.activation(dum_sb[:], dum_sb[:], Act.Sign)
            for k, ch in enumerate(DVE_ORDER):
                scalar.wait_ge(ar_sem, k + 1)
                o = ch * CW
                scalar.activation(
                    d_sb[:, o : o + CW], sub_sb[:, o : o + CW], Act.Sign
                ).then_inc(ad_sem, 1)
            scalar.wait_ge(alr_sem, 1)
            scalar.activation(dl_sb[:], subl_sb[:], Act.Sign).then_inc(ad_sem, 1)

        @block.tensor
        def _(tensor):
            tensor.wait_ge(g_sem, 1)
            for _ in range(N_WARM):
                nc.tensor.matmul(psw[:], warm_sb[:], warm_sb[:], start=True, stop=True)

            n0 = n1 = 0  # issued counts for psa0 (c0,c2) / psa1 (c1)
            N0 = 2 * NCH * TPC
            N1 = NCH * TPC

            def psa_chunk(ch):
                nonlocal n0, n1
                tensor.wait_ge(t_sems[ch], 16)
                tensor.wait_ge(p_sems[ch], 16)
                last = None
                for c in range(C):
                    for i in range(TPC):
                        if c == 1:
                            mm = nc.tensor.matmul(
                                psa1[:], ptile(ch, c, i), tblk(ch, c, i),
                                start=(n1 == 0), stop=(n1 == N1 - 1),
                            )
                            n1 += 1
                        else:
                            mm = nc.tensor.matmul(
                                psa0[:], ptile(ch, c, i), tblk(ch, c, i),
                                start=(n0 == 0), stop=(n0 == N0 - 1),
                            )
                            n0 += 1
                        last = mm
                return last

            nd = 0

            def psc_chunk(ch, sig_rank):
                nonlocal nd
                tensor.wait_ge(ad_sem, sig_rank)
                last = None
                for i in range(TPC):
                    last = nc.tensor.matmul(
                        psc[:], ptile(ch, 1, i), dtile(ch, i),
                        start=(nd + i == 0), stop=(nd + i == NCH * TPC - 1),
                    )
                nd += TPC
                return last

            # t arrival order 0,1 (pool), then 2 (sync), 3 (scalar).
            psa_chunk(0)
            psa_chunk(1)
            # d chunks complete in DVE_ORDER; early psc squeezed between psa
            psc_chunk(DVE_ORDER[0], 1)
            psc_chunk(DVE_ORDER[1], 2)
            psa_chunk(2)
            psa_chunk(3).then_inc(pe_sem, 1)
            psc_chunk(DVE_ORDER[2], 3)
            psc_chunk(DVE_ORDER[3], 4).then_inc(pe_sem, 1)

    return nc


def _shard(probs, target):
    """f32 [B,C,D,H,W] x2 -> per-core fp8 arrays p/t/aux (see module doc)."""
    pf = np.ascontiguousarray(probs.transpose(1, 0, 2, 3, 4)).reshape(C, N_VOX)
    tf = np.ascontiguousarray(target.transpose(1, 0, 2, 3, 4)).reshape(C, N_VOX)
    out = []
    for k in range(N_CORES):
        sl = slice(k * VOX_PER_CORE, (k + 1) * VOX_PER_CORE)
        # [C, P, NCH, CW]
        pk = pf[:, sl].reshape(C, P, NCH, CW).astype(F8)
        tk = tf[:, sl].reshape(C, P, NCH, CW).astype(F8)
        ph = np.empty((NCH, P, PW), dtype=F8)
        th = np.empty((NCH, P, TW), dtype=F8)
        aux = np.empty((P, 2 * NLOST), dtype=F8)
        one = np.ones((P,), dtype=F8)
        for ch in range(NCH):
            for c in range(C):
                for i in range(TPC):
                    tile_p = pk[c, :, ch, i * P : (i + 1) * P]
                    tile_t = tk[c, :, ch, i * P : (i + 1) * P]
                    po = (c * TPC + i) * P
                    ph[ch, :, po : po + P - 1] = tile_p[:, 0 : P - 1]
                    ph[ch, :, po + P - 1] = one
                    to = (c * TPC + i) * (P + 1)
                    th[ch, :, to : to + P] = tile_t
                    th[ch, :, to + P] = one
                    g = ch * TPC + i
                    aux[:, c * NT + g] = tile_p[:, P - 1]
                    aux[:, NLOST + c * NT + g] = tile_t[:, P - 1]
        out.append({"p": ph, "t": th, "aux": aux})
    return out


def _finalize(results):
    tp = sp = st = sp1 = s1p = 0.0
    for r in results:
        fin = r["fin"].astype(np.float64)
        a0 = fin[:, 0:129]
        a1 = fin[:, 129:258]
        c_ = fin[:, 258:386]
        m = np.arange(127)
        tp += a0[m, m].sum() + a1[m, m].sum() + fin[:, 386].sum()
        sp += a0[:127, 128].sum() + a1[:127, 128].sum() + fin[:, 387].sum()
        sp1 += a1[:127, 128].sum() + fin[:, 388].sum()
        st += a0[127, 0:128].sum() + a1[127, 0:128].sum()
        s1p += c_[m, m].sum() + fin[:, 389].sum()
    s1_sum = 0.5 * (sp1 + s1p)
    surface = s1_sum / float(N_VOX)
    tversky = 1.0 - (tp + 1.0) / (0.5 * (sp + st) + 1.0)
    return np.float32(surface + tversky)


def kernel(probs: np.ndarray, target: np.ndarray) -> np.ndarray:
    probs = np.asarray(probs, dtype=np.float32)
    target = np.asarray(target, dtype=np.float32)

    if "nc" not in _CACHE:
        _CACHE["nc"] = _build_module()
    nc = _CACHE["nc"]

    in_maps = _shard(probs, target)
    res = run_bass_kernel_spmd(nc, in_maps, core_ids=list(range(N_CORES)))
    return _finalize(res.results)
